# revision 52
# baseline (speedup 1.0000x reference)
"""Trainium2 Bass kernel for the annular photonic transfer-matrix reflectance
sweep (W=2097152 wavelengths, L=6 layers), data-parallel over 8 NeuronCores.

Quadrature-fit formulation. For each shell l (radii r0, r1 baked at build
time) every transfer-matrix entry is written as

    entry(t) = P(w)*cos(D) +/- Q(w)*sin(D),   w = 1/t,  D = t*(r1-r0),

where P, Q are degree-3 polynomials in w fitted at kernel() time against
the float64 Numerical-Recipes Bessel evaluation (the reference's own
definition) and factored into lin(w)*[(w+a)^2+b] so each evaluates as
ACT-Copy + ACT-Square + one Vector STT. The per-layer impedance factors
p_l = t_l/omega are folded away exactly: the similarity transform
diag(1, omega) telescopes through the chain product and 1/omega cancels
in the reflectance ratio, so the kernel never computes p_l, 1/p_l, or any
per-shell reciprocal. Boundary C-factors are two fitted polys per
boundary times t.  D is range-reduced with a 1-term Cody-Waite (k <= 6),
cos via Sin(pi/2 - |x|).

Engine plan (measured costs in the project memory): Scalar/ACT runs all
sqrt/ln/exp/affine/sin/abs/square work with activation tables grouped
sqrt -> ln+exp -> trig; Vector runs the STTs plus the entire fp16
tensor_tensor block (applies, entries, chain, final) in the 2x packed
mode; GpSimd is left idle because its SBUF-port traffic slows the DVE
~3.5x. Shell fits are scaled by 0.25 for fp16 range headroom (cancels in
the ratio). Per chunk, a prologue computes t, w16 = 1/t (fp16), t16 and
the rescaled phase residue u = t - k*(2pi/delta) (so the body gets
sin/cos of delta*u via the Sin activation's scale slot - no Delta
tensor, 1-term reduction since k <= 6); the Vector-heavy body runs
trig + polys + chain + final.  Emission order P0 B0 P1 B1 overlaps
Scalar's prologue-1 with Vector's body-0, and chunk-0's flexible
affines run on the otherwise-idle Vector head.  The kernel outputs
ren/imn/red/imd (fp16); the host computes
R = (ren^2+imn^2)/(red^2+imd^2).
"""
import numpy as np

import bass_rust
import concourse.bass as bass
import concourse.tile as tile
from concourse import mybir
from concourse.vector_clock import ScopedClock

F32 = mybir.dt.float32
AL = mybir.AluOpType
AF = mybir.ActivationFunctionType

W = 2097152
L = 6
NCORES = 8
P = 128
WS = W // NCORES          # 262144 elements per core
FT = WS // P              # 2048 free dim per core
FC = 1024                 # chunk of free dim processed at once
NSLOTS = 21               # f32 churn work tiles
NSLOTS16 = 37             # fp16 churn work tiles
NKEEP = 24                # fp16 long-lived tiles (both chunks' w16/t16)
DEG = 3                   # polynomial degree of the quadrature fits

PI = float(np.pi)
TWO_OVER_PI = 0.636619772

# ---- Cody-Waite split of 2*pi ------------------------------------------------


def _split_const(v, bits=12):
    u = np.uint32(np.float32(v).view(np.uint32))
    mask = np.uint32(0xFFFFFFFF) << np.uint32(23 - bits + 1)
    return float(np.uint32(u & mask).view(np.float32))


_2PI = 2 * PI
CW_C1 = _split_const(_2PI)
CW_C2 = _split_const(_2PI - CW_C1)
INV_2PI = float(np.float32(1.0 / _2PI))
MAGIC = 8388608.0

# ---- NR Bessel coefficients (highest degree first) ---------------------------

J0_NUM = [-184.9052456, 77392.33017, -11214424.18, 651619640.7,
          -13362590354.0, 57568490574.0]
J0_DEN = [1.0, 267.8532712, 59272.64853, 9494680.718,
          1029532985.0, 57568490411.0]
J1_NUM = [-30.16036606, 15704.48260, -2972611.439, 242396853.1,
          -7895059235.0, 72362614232.0]
J1_DEN = [1.0, 376.9991397, 99447.43394, 18583304.74,
          2300535178.0, 144725228442.0]
Y0_NUM = [228.4622733, -86327.92757, 10879881.29, -512359803.6,
          7062834065.0, -2957821389.0]
Y0_DEN = [1.0, 226.1030244, 47447.26470, 7189466.438,
          745249964.8, 40076544269.0]
Y1_NUM = [8.511937935e4, -4.237922726e7, 7.349264551e9,
          -5.153438139e11, 1.275274390e13, -4.900604943e13]
Y1_DEN = [1.0, 3.549632885e3, 1.020426050e6, 2.245904002e8,
          3.733650367e10, 4.244419664e12, 2.499580570e14]
P0C = [0.2093887211e-6, -0.2073370639e-5, 0.2734510407e-4,
       -0.1098628627e-2, 1.0]
Q0C = [-0.934935152e-7, 0.7621095161e-6, -0.6911147651e-5,
       0.1430488765e-3, -0.1562499995e-1]
P1C = [-0.240337019e-6, 0.2457520174e-5, -0.3516396496e-4,
       0.183105e-2, 1.0]
Q1C = [0.105787412e-6, -0.88228987e-6, 0.8449199096e-5,
       -0.2002690873e-3, 0.04687499995]

# ---- float64 NR Bessel (fit targets; matches reference.py's definition) ------


def _poly64(y, c):
    acc = np.full_like(y, c[0])
    for cc in c[1:]:
        acc = acc * y + cc
    return acc


def _j0_64(x):
    y = x * x
    small = _poly64(y, J0_NUM) / _poly64(y, J0_DEN)
    z = 8.0 / x
    y2 = z * z
    xx = x - 0.785398164
    big = np.sqrt(TWO_OVER_PI / x) * (np.cos(xx) * _poly64(y2, P0C)
                                      - z * np.sin(xx) * _poly64(y2, Q0C))
    return np.where(x < 8.0, small, big)


def _j1_64(x):
    y = x * x
    small = x * _poly64(y, J1_NUM) / _poly64(y, J1_DEN)
    z = 8.0 / x
    y2 = z * z
    xx = x - 2.356194491
    big = np.sqrt(TWO_OVER_PI / x) * (np.cos(xx) * _poly64(y2, P1C)
                                      - z * np.sin(xx) * _poly64(y2, Q1C))
    return np.where(x < 8.0, small, big)


def _y0_64(x):
    y = x * x
    small = _poly64(y, Y0_NUM) / _poly64(y, Y0_DEN) \
        + TWO_OVER_PI * _j0_64(x) * np.log(x)
    z = 8.0 / x
    y2 = z * z
    xx = x - 0.785398164
    big = np.sqrt(TWO_OVER_PI / x) * (np.sin(xx) * _poly64(y2, P0C)
                                      + z * np.cos(xx) * _poly64(y2, Q0C))
    return np.where(x < 8.0, small, big)


def _y1_64(x):
    y = x * x
    small = x * _poly64(y, Y1_NUM) / _poly64(y, Y1_DEN) \
        + TWO_OVER_PI * (_j1_64(x) * np.log(x) - 1.0 / x)
    z = 8.0 / x
    y2 = z * z
    xx = x - 2.356194491
    big = np.sqrt(TWO_OVER_PI / x) * (np.sin(xx) * _poly64(y2, P1C)
                                      + z * np.cos(xx) * _poly64(y2, Q1C))
    return np.where(x < 8.0, small, big)


def _norm_parts(x):
    """C0,S0 = m0*(cos f0, sin f0); C1,S1 = m1*(cos f1, sin f1): the
    amplitude-normalized Bessel values rotated by the asymptotic phase."""
    amp = np.sqrt(PI * x / 2)
    nJ0, nY0 = amp * _j0_64(x), amp * _y0_64(x)
    nJ1, nY1 = amp * _j1_64(x), amp * _y1_64(x)
    c = np.cos(x - PI / 4)
    s = np.sin(x - PI / 4)
    C0 = nJ0 * c + nY0 * s
    S0 = nY0 * c - nJ0 * s
    c3 = np.cos(x - 3 * PI / 4)
    s3 = np.sin(x - 3 * PI / 4)
    C1 = nJ1 * c3 + nY1 * s3
    S1 = nY1 * c3 - nJ1 * s3
    return C0, S0, C1, S1, nJ0, nY0, nJ1, nY1


def _shell_targets(w, r0, r1):
    x0 = r0 / w
    x1 = r1 / w
    C0a, S0a, C1a, S1a = _norm_parts(x0)[:4]
    C0b, S0b, C1b, S1b = _norm_parts(x1)[:4]
    return dict(
        Pa=C1a * C0b + S1a * S0b, Qa=S0b * C1a - C0b * S1a,
        Pb=w * (C0a * C0b + S0a * S0b), Qb=w * (S0b * C0a - C0b * S0a),
        Pc=C1a * C1b + S1a * S1b, Qc=S1b * C1a - C1b * S1a,
        Pd=C0a * C1b + S0a * S1b, Qd=S1b * C0a - C1b * S0a,
    )


def _boundary_targets(w, r):
    x = r / w
    _, _, _, _, nJ0, nY0, nJ1, nY1 = _norm_parts(x)
    m0sq = nJ0 * nJ0 + nY0 * nY0
    return dict(Fs=-(nJ1 * nJ0 + nY1 * nY0) / m0sq,
                Fc=-(nJ1 * nY0 - nY1 * nJ0) / m0sq)


def _cheb_grid(lo, hi, n):
    k = np.arange(n)
    x = np.cos(PI * (k + 0.5) / n)
    return lo + (hi - lo) * (x + 1) / 2


def _fit(w, y, deg):
    V = np.vander(w, deg + 1)
    coef, *_ = np.linalg.lstsq(V, y, rcond=None)
    return [float(c) for c in coef]


def _plan3(coeffs, dense, target, scale=1.0):
    """Factor the fitted cubic: p = lin(w) * [(w+a)^2 + b] with
    lin = lin_s*w + lin_b; `scale` multiplies the fitted function (fp16
    range headroom; cancels in the final ratio)."""
    if abs(coeffs[0]) < 1e-9 * max(abs(c) for c in coeffs):
        return None
    roots = np.roots(coeffs)
    if len(roots) != 3:
        return None
    ii = int(np.argmin(np.abs(roots.imag)))
    rreal = float(roots[ii].real)
    rest = [roots[j] for j in range(3) if j != ii]
    a = float(-0.5 * (rest[0] + rest[1]).real)
    b = float((rest[0] * rest[1]).real
              - 0.25 * ((rest[0] + rest[1]).real) ** 2)
    c0 = coeffs[0] * scale
    lin_s, lin_b = c0, -c0 * rreal
    f = np.float32
    w = dense.astype(np.float32)
    lin = f(f(lin_s) * w + f(lin_b))
    sq = f(f(w + f(a)) ** 2)
    v = f(f(sq + f(b)) * lin)
    if np.abs(v.astype(np.float64) - target * scale).max() / scale > 1.3e-3:
        return None
    return {'form': '3', 'lin_s': float(lin_s), 'lin_b': float(lin_b),
            'a': float(a), 'b': float(b)}


def _plan4(coeffs, dense, target, scale=1.0):
    """Degree-4 Horner fallback (scale folded into the coefficients)."""
    return {'form': 'A', 'coeffs': [c * scale for c in coeffs]}


SHELL_SCALE = 0.25   # per-shell fit scale; chain scales by 0.25^4, ratio-safe


def fit_all(rho64, wlo, whi, n=2500):
    nodes = _cheb_grid(wlo, whi, n)
    dense = np.linspace(wlo, whi, 8011)
    fits = {}
    for l in range(1, L - 1):
        t = _shell_targets(nodes, rho64[l, 0], rho64[l, 1])
        td = _shell_targets(dense, rho64[l, 0], rho64[l, 1])
        fits[l] = {}
        for k in t:
            p = _plan3(_fit(nodes, t[k], 3), dense, td[k], SHELL_SCALE)
            if p is None:
                p = _plan4(_fit(nodes, t[k], 4), dense, td[k], SHELL_SCALE)
            fits[l][k] = p
    for (l, r) in ((0, rho64[0, 1]), (L - 1, rho64[L - 1, 0])):
        t = _boundary_targets(nodes, r)
        td = _boundary_targets(dense, r)
        fits[l] = {}
        for k in t:
            p = _plan3(_fit(nodes, t[k], 3), dense, td[k], 1.0)
            if p is None:
                p = _plan4(_fit(nodes, t[k], 4), dense, td[k], 1.0)
            fits[l][k] = p
    return fits


# ---- walrus 1-sync-wait-per-instruction workaround --------------------------
_MAXW = 1


def _split_waits(nc):
    for f in nc.m.functions:
        for bb in f.blocks:
            arr = list(bb.instructions)
            out = []
            changed = False
            for mi in arr:
                si = mi.sync_info
                waits = list(si.on_wait) if si is not None and si.on_wait else []
                if len(waits) > _MAXW:
                    changed = True
                    upd = list(si.on_update) if si is not None and si.on_update \
                        else []
                    rest = waits[_MAXW:]
                    for i in range(0, len(rest), _MAXW):
                        ev = nc.engines[mi.engine].nop()
                        cur = nc.cur_bb.bb
                        cur.instructions = [
                            x for x in cur.instructions if x.name != ev.ins.name
                        ]
                        ev.ins.sync_info = bass_rust.SyncInfo(
                            on_wait=rest[i:i + _MAXW], on_update=[])
                        out.append(ev.ins)
                    mi.sync_info = bass_rust.SyncInfo(on_wait=waits[:_MAXW],
                                                      on_update=upd)
                out.append(mi)
            if changed:
                bb.instructions = out


def _patched_drain_and_barrier(self, tick_clock, wait_clock):
    nc = self.nc
    drain_inst = nc.sync.drain()
    wait_clock.add_sem_waits(
        drain_inst.ins, ScopedClock({None: tick_clock.global_clock})
    )
    nc.all_engine_barrier()
    assert self.sems is not None
    popped = nc._tile_sem_poison_stack.pop()
    assert popped is self._sem_poison
    nc.clear_and_free_semaphores(list(self.sems.allocated().values()))
    nc.all_engine_barrier()


tile.TileContext._drain_and_barrier = _patched_drain_and_barrier


def _register_const(nc, *values):
    for v in values:
        v = float(v)
        if (F32, v) in nc.const_aps.aps:
            continue
        t = nc.alloc_sbuf_tensor(f"const-f32-{v}", [128, 1], F32)
        nc.gpsimd.memset(t.ap(), v)
        nc.const_aps.aps[(F32, v)] = t.ap()
    nc.all_engine_barrier()


# ---- kernel emitter ----------------------------------------------------------

# GpSimd shares SBUF ports with the DVE: concurrent Pool tensor_tensor
# slows Vector ~3.5x (measured), so Pool gets no elementwise work at all.
F16 = mybir.dt.float16


def _poly_biases(plan):
    """Square-activation bias constants a plan needs (pre-registered)."""
    if plan['form'] == '3':
        return [float(plan['a'])]
    return []


class Emit:
    def __init__(self, nc, pool):
        self.nc = nc
        self.pool = pool
        self.n = 0

    def t(self):
        self.n += 1
        return self.pool.tile([P, FC], F32, name=f"w{self.n}", tag="w",
                              bufs=NSLOTS)

    def t16(self):
        self.n += 1
        return self.pool.tile([P, FC], F16, name=f"h{self.n}", tag="h",
                              bufs=NSLOTS16)

    def k16(self):
        self.n += 1
        return self.pool.tile([P, FC], F16, name=f"k{self.n}", tag="k",
                              bufs=NKEEP)

    def act(self, a, func, bias=0.0, scale=1.0, out=None):
        out = out if out is not None else self.t()
        self.nc.scalar.activation(out[:], a[:], func, float(bias), float(scale))
        return out

    def affine(self, a, scale, bias):
        return self.act(a, AF.Copy, bias, scale)

    def stt(self, a, s, b, op0, op1, out=None):
        out = out if out is not None else self.t()
        self.nc.vector.scalar_tensor_tensor(out[:], a[:], float(s), b[:],
                                            AL[op0], AL[op1])
        return out

    def tt(self, a, b, op, out=None):
        out = out if out is not None else self.t()
        self.nc.vector.tensor_tensor(out[:], a[:], b[:], AL[op])
        return out

    def tt16(self, a, b, op):
        return self.tt(a, b, op, out=self.t16())

    def poly16(self, w16, plan, mult16):
        """Fitted poly at w16 (fp16) times mult16 (fp16) -> fp16."""
        if plan['form'] == 'A':
            coeffs = plan['coeffs']
            acc = self.act(w16, AF.Copy, 0.0, coeffs[0])
            for c in coeffs[1:-1]:
                acc = self.stt(acc, c, w16, "add", "mult")
            return self.stt(acc, coeffs[-1], mult16, "add", "mult",
                            out=self.t16())
        lin = self.affine(w16, plan['lin_s'], plan['lin_b'])
        sq = self.act(w16, AF.Square, plan['a'], 1.0)
        v = self.stt(sq, plan['b'], lin, "add", "mult", out=self.t16())
        return self.tt16(v, mult16, "mult")


def build(rho32, fits):
    """rho32: float32 [L,2]; fits: per-layer plan dict."""
    nc = bass.Bass()
    consts = [0.0, PI / 2]
    for l in fits:
        for plan in fits[l].values():
            consts.extend(_poly_biases(plan))
    _register_const(nc, *consts)
    om_d = nc.declare_dram_parameter("omega", [P, FT], F32, isOutput=False)
    ek_d = nc.declare_dram_parameter("epsk", [L, P, FT], F32, isOutput=False)
    out_d = {name: nc.declare_dram_parameter(name, [P, FT], F16, isOutput=True)
             for name in ("ren", "imn", "red", "imd")}

    deltas = [float(np.float32(rho32[l, 1]) - np.float32(rho32[l, 0]))
              for l in range(L)]

    FL2PI = float(np.float32(2 * np.pi))
    NCH = FT // FC
    with tile.TileContext(nc) as tc:
        with tc.tile_pool(name="work", bufs=NSLOTS) as pool:
            em = Emit(nc, pool)

            # ---- prologue (ACT-heavy; tables: sqrt -> ln/exp -> trig).
            # For chunk 0 the flexible affines (t16 copy, tr) run on the
            # otherwise-idle Vector engine to shrink the Scalar-bound head;
            # later chunks keep them on Scalar, which idles at the tail. ----
            def prologue(ci):
                on_v = (ci == 0)
                sl = slice(ci * FC, (ci + 1) * FC)
                omega = em.t()
                nc.sync.dma_start(omega[:], om_d[:, sl])
                eps = []
                for l in range(L):
                    e = em.t()
                    nc.sync.dma_start(e[:], ek_d[l, :, sl])
                    eps.append(e)
                se = [em.act(eps[l], AF.Sqrt) for l in range(L)]
                t = [em.tt(omega, se[l], "mult") for l in range(L)]
                lnt = [em.act(t[l], AF.Ln) for l in range(L)]
                w16 = [em.act(lnt[l], AF.Exp, 0.0, -1.0, out=em.k16())
                       for l in range(L)]
                if on_v:
                    t16 = [em.tt(omega, se[l], "mult", out=em.k16())
                           for l in range(L)]
                else:
                    t16 = [em.act(t[l], AF.Copy, out=em.k16())
                           for l in range(L)]
                # rescaled range reduction: u = t - k*(2pi/delta) with
                # k = round(t*delta/2pi); the body computes sin/cos of
                # delta*u via the Sin activation's scale slot.
                u = {}
                for l in range(1, L - 1):
                    dl = deltas[l]
                    if on_v:
                        tr = em.t()
                        nc.vector.tensor_scalar(tr[:], t[l][:], dl * INV_2PI,
                                                MAGIC, AL["mult"], AL["add"])
                    else:
                        tr = em.affine(t[l], dl * INV_2PI, MAGIC)
                    kf2 = em.t()
                    nc.vector.tensor_scalar(kf2[:], tr[:], -MAGIC,
                                            FL2PI / dl, AL["add"], AL["mult"])
                    u[l] = em.tt(t[l], kf2, "subtract")
                return dict(sl=sl, w16=w16, t16=t16, u=u)

            # ---- body (Vector-heavy fp16 block; its ACT ops are Sin once
            # per shell plus Copy/Square, which live in every table) ----
            def body(p, mid=None):
                w16, t16, u = p['w16'], p['t16'], p['u']

                def shell(l):
                    c = fits[l]
                    dl = deltas[l]
                    SD = em.act(u[l], AF.Sin, 0.0, dl, out=em.t16())
                    au = em.act(u[l], AF.Abs)
                    CD = em.act(au, AF.Sin, PI / 2, -dl, out=em.t16())
                    specs = [('Pa', CD), ('Qa', SD), ('Pb', SD),
                             ('Qb', CD), ('Pc', SD), ('Qc', CD),
                             ('Pd', CD), ('Qd', SD)]
                    term = {}
                    for k, trig in specs:
                        term[k] = em.poly16(w16[l], c[k], trig)
                    a = em.tt16(term['Pa'], term['Qa'], "subtract")
                    bt = em.tt16(term['Pb'], term['Qb'], "add")
                    cpre = em.tt16(term['Pc'], term['Qc'], "add")
                    ct = em.tt16(cpre, t16[l], "mult")
                    dd2 = em.tt16(term['Pd'], term['Qd'], "subtract")
                    return a, bt, ct, dd2

                # boundary polys depend only on prologue outputs - hoist
                # them ahead of the chain so the tail is just chain+final
                Ur = em.poly16(w16[0], fits[0]['Fs'], t16[0])
                Ui = em.poly16(w16[0], fits[0]['Fc'], t16[0])
                Vr = em.poly16(w16[L - 1], fits[L - 1]['Fs'], t16[L - 1])
                Vi = em.poly16(w16[L - 1], fits[L - 1]['Fc'], t16[L - 1])

                A, B, C, D = shell(1)
                for l in (2, 3, 4):
                    if l == 3 and mid is not None:
                        mid()      # interleave next chunk's prologue here
                    a, bt, ct, dd2 = shell(l)
                    m1 = em.tt16(A, a, "mult")
                    m2 = em.tt16(B, ct, "mult")
                    A2 = em.tt16(m1, m2, "subtract")
                    m3 = em.tt16(A, bt, "mult")
                    m4 = em.tt16(B, dd2, "mult")
                    B2 = em.tt16(m3, m4, "add")
                    m5 = em.tt16(C, a, "mult")
                    m6 = em.tt16(D, ct, "mult")
                    C2 = em.tt16(m5, m6, "add")
                    m7 = em.tt16(D, dd2, "mult")
                    m8 = em.tt16(C, bt, "mult")
                    D2 = em.tt16(m7, m8, "subtract")
                    A, B, C, D = A2, B2, C2, D2

                # 18-op final: num/den via shared conjugate products
                e_ = em.tt16(Ur, B, "mult")
                f_ = em.tt16(Ui, B, "mult")
                G1r = em.tt16(D, e_, "subtract")
                pa = em.tt16(Vr, G1r, "mult")
                pb = em.tt16(Vi, f_, "mult")
                pc = em.tt16(Vi, G1r, "mult")
                pd = em.tt16(Vr, f_, "mult")
                reVG1 = em.tt16(pa, pb, "add")
                imVG1 = em.tt16(pc, pd, "subtract")
                reVG2 = em.tt16(pa, pb, "subtract")
                imVG2 = em.tt16(pc, pd, "add")
                ua = em.tt16(Ur, A, "mult")
                ub = em.tt16(Ui, A, "mult")
                s1 = em.tt16(C, ua, "add")
                ren = em.tt16(s1, reVG1, "subtract")
                imn = em.tt16(ub, imVG1, "subtract")
                red = em.tt16(s1, reVG2, "subtract")
                imd = em.tt16(ub, imVG2, "add")
                for name, v in (("ren", ren), ("imn", imn),
                                ("red", red), ("imd", imd)):
                    nc.sync.dma_start(out_d[name][:, p['sl']], v[:])

            # P0 B0 P1 B1: Vector enters body-0 as soon as prologue-0 is
            # done while Scalar runs ahead into prologue-1.
            for ci in range(NCH):
                body(prologue(ci))
    _split_waits(nc)
    return nc


# ---- host-side entry ---------------------------------------------------------

_CACHE = {}
TRACE = False
LAST_RESULT = None


def _numpy_fallback(omega, eps, mu, rho):
    """Exact reference math in numpy (mu != 1 path only)."""

    def poly(y, coeffs):
        acc = np.full_like(y, np.float32(coeffs[0]))
        for c2 in coeffs[1:]:
            acc = acc * y + np.float32(c2)
        return acc

    def _j0(x):
        y = x * x
        small = poly(y, J0_NUM) / poly(y, J0_DEN)
        z = np.float32(8.0) / x
        y2 = z * z
        xx = x - np.float32(0.785398164)
        big = np.sqrt(np.float32(TWO_OVER_PI) / x) * (
            np.cos(xx) * poly(y2, P0C) - z * np.sin(xx) * poly(y2, Q0C))
        return np.where(x < 8.0, small, big).astype(np.float32)

    def _j1(x):
        y = x * x
        small = x * poly(y, J1_NUM) / poly(y, J1_DEN)
        z = np.float32(8.0) / x
        y2 = z * z
        xx = x - np.float32(2.356194491)
        big = np.sqrt(np.float32(TWO_OVER_PI) / x) * (
            np.cos(xx) * poly(y2, P1C) - z * np.sin(xx) * poly(y2, Q1C))
        return np.where(x < 8.0, small, big).astype(np.float32)

    def _y0(x):
        y = x * x
        small = poly(y, Y0_NUM) / poly(y, Y0_DEN) + \
            np.float32(TWO_OVER_PI) * _j0(x) * np.log(x)
        z = np.float32(8.0) / x
        y2 = z * z
        xx = x - np.float32(0.785398164)
        big = np.sqrt(np.float32(TWO_OVER_PI) / x) * (
            np.sin(xx) * poly(y2, P0C) + z * np.cos(xx) * poly(y2, Q0C))
        return np.where(x < 8.0, small, big).astype(np.float32)

    def _y1(x):
        y = x * x
        small = x * poly(y, Y1_NUM) / poly(y, Y1_DEN) + \
            np.float32(TWO_OVER_PI) * (_j1(x) * np.log(x) - 1.0 / x)
        z = np.float32(8.0) / x
        y2 = z * z
        xx = x - np.float32(2.356194491)
        big = np.sqrt(np.float32(TWO_OVER_PI) / x) * (
            np.sin(xx) * poly(y2, P1C) + z * np.cos(xx) * poly(y2, Q1C))
        return np.where(x < 8.0, small, big).astype(np.float32)

    omega = omega.astype(np.float32)
    eps = eps.astype(np.float32)
    mu = mu.astype(np.float32)
    k = omega[None, :] * np.sqrt(eps * mu)
    p = np.sqrt(eps / mu)

    def tmat(kl, pl, r0, r1):
        x0 = kl * np.float32(r0)
        x1 = kl * np.float32(r1)
        j_a, y_a = _j0(x0), _y0(x0)
        j_b, y_b = _j0(x1), _y0(x1)
        jd_a, yd_a = -_j1(x0), -_y1(x0)
        jd_b, yd_b = -_j1(x1), -_y1(x1)
        pref = np.float32(PI / 2) * x0
        m00 = (pref * (yd_a * j_b - jd_a * y_b)).astype(np.complex64)
        m01 = (1j / pl) * pref * (j_a * y_b - y_a * j_b)
        m10 = (-1j * pl) * pref * (yd_a * jd_b - jd_a * yd_b)
        m11 = (pref * (j_a * yd_b - y_a * jd_b)).astype(np.complex64)
        return m00, m01, m10, m11

    M00, M01, M10, M11 = tmat(k[1], p[1], rho[1, 0], rho[1, 1])
    for l in range(2, L - 1):
        a, b, c, d = tmat(k[l], p[l], rho[l, 0], rho[l, 1])
        M00, M01, M10, M11 = (M00 * a + M01 * c, M00 * b + M01 * d,
                              M10 * a + M11 * c, M10 * b + M11 * d)

    def cfacs(z):
        j0v, j1v, y0v, y1v = _j0(z), _j1(z), _y0(z), _y1(z)
        c1 = -(j1v + 1j * y1v) / (j0v + 1j * y0v)
        c2 = -(j1v - 1j * y1v) / (j0v - 1j * y0v)
        return c1, c2

    c0_1, c0_2 = cfacs(k[0] * np.float32(rho[0, 1]))
    _, c1_2 = cfacs(k[L - 1] * np.float32(rho[L - 1, 0]))
    p0, p1 = p[0], p[L - 1]
    num = M10 + 1j * p0 * c0_2 * M00 \
        - 1j * p1 * c1_2 * (M11 + 1j * p0 * c0_2 * M01)
    den = -1j * p0 * c0_1 * M00 - M10 \
        - 1j * p1 * c1_2 * (-1j * p0 * c0_1 * M01 - M11)
    r = num / den
    return (r * np.conj(r)).real.astype(np.float32)


def kernel(omega, eps, mu, rho):
    from concourse.bass_utils import run_bass_kernel_spmd

    omega = np.ascontiguousarray(omega, dtype=np.float32)
    eps = np.ascontiguousarray(eps, dtype=np.float32)
    mu = np.ascontiguousarray(mu, dtype=np.float32)
    rho32 = np.asarray(rho, dtype=np.float32)
    assert omega.shape == (W,) and eps.shape == (L, W)

    if not bool(np.all(mu == 1.0)):
        return _numpy_fallback(omega, eps, mu, rho32)

    # fit range from input bounds (w = 1/t, t = omega*sqrt(eps))
    om_min, om_max = float(omega.min()), float(omega.max())
    ep_min, ep_max = float(eps.min()), float(eps.max())
    tmin = om_min * np.sqrt(ep_min)
    tmax = om_max * np.sqrt(ep_max)
    wlo = float(1.0 / (tmax * 1.002))
    whi = float(1.0 / (tmin * 0.998))

    key = (rho32.tobytes(), round(wlo, 5), round(whi, 5), DEG, FC, NSLOTS,
           NSLOTS16, NKEEP, SHELL_SCALE, "v16")
    if key not in _CACHE:
        fits = fit_all(rho32.astype(np.float64), wlo, whi)
        _CACHE[key] = build(rho32, fits)
    nc = _CACHE[key]

    in_maps = []
    for i in range(NCORES):
        sl = slice(i * WS, (i + 1) * WS)
        in_maps.append({"omega": omega[sl].reshape(P, FT),
                        "epsk": eps[:, sl].reshape(L, P, FT)})

    res = run_bass_kernel_spmd(nc, in_maps, core_ids=list(range(NCORES)),
                               trace=TRACE)
    global LAST_RESULT
    LAST_RESULT = res
    out = np.empty((W,), dtype=np.float32)
    for i in range(NCORES):
        r = res.results[i]
        ren = r["ren"].reshape(WS).astype(np.float32)
        imn = r["imn"].reshape(WS).astype(np.float32)
        red = r["red"].reshape(WS).astype(np.float32)
        imd = r["imd"].reshape(WS).astype(np.float32)
        out[i * WS:(i + 1) * WS] = (ren * ren + imn * imn) / \
            (red * red + imd * imd)
    return out


# revision 53
# speedup vs baseline: 1.0008x; 1.0008x over previous
"""Trainium2 Bass kernel for the annular photonic transfer-matrix reflectance
sweep (W=2097152 wavelengths, L=6 layers), data-parallel over 8 NeuronCores.

Quadrature-fit formulation. For each shell l (radii r0, r1 baked at build
time) every transfer-matrix entry is written as

    entry(t) = P(w)*cos(D) +/- Q(w)*sin(D),   w = 1/t,  D = t*(r1-r0),

where P, Q are degree-3 polynomials in w fitted at kernel() time against
the float64 Numerical-Recipes Bessel evaluation (the reference's own
definition) and factored into lin(w)*[(w+a)^2+b] so each evaluates as
ACT-Copy + ACT-Square + one Vector STT. The per-layer impedance factors
p_l = t_l/omega are folded away exactly: the similarity transform
diag(1, omega) telescopes through the chain product and 1/omega cancels
in the reflectance ratio, so the kernel never computes p_l, 1/p_l, or any
per-shell reciprocal. Boundary C-factors are two fitted polys per
boundary times t.  D is range-reduced with a 1-term Cody-Waite (k <= 6),
cos via Sin(pi/2 - |x|).

Engine plan (measured costs in the project memory): Scalar/ACT runs all
sqrt/ln/exp/affine/sin/abs/square work with activation tables grouped
sqrt -> ln+exp -> trig; Vector runs the STTs plus the entire fp16
tensor_tensor block (applies, entries, chain, final) in the 2x packed
mode; GpSimd is left idle because its SBUF-port traffic slows the DVE
~3.5x. Shell fits are scaled by 0.25 for fp16 range headroom (cancels in
the ratio). Per chunk, a prologue computes t, w16 = 1/t (fp16), t16 and
the rescaled phase residue u = t - k*(2pi/delta) (so the body gets
sin/cos of delta*u via the Sin activation's scale slot - no Delta
tensor, 1-term reduction since k <= 6); the Vector-heavy body runs
trig + polys + chain + final.  Emission order P0 B0 P1 B1 overlaps
Scalar's prologue-1 with Vector's body-0, and chunk-0's flexible
affines run on the otherwise-idle Vector head.  The kernel outputs
ren/imn/red/imd (fp16); the host computes
R = (ren^2+imn^2)/(red^2+imd^2).
"""
import numpy as np

import bass_rust
import concourse.bass as bass
import concourse.tile as tile
from concourse import mybir
from concourse.vector_clock import ScopedClock

F32 = mybir.dt.float32
AL = mybir.AluOpType
AF = mybir.ActivationFunctionType

W = 2097152
L = 6
NCORES = 8
P = 128
WS = W // NCORES          # 262144 elements per core
FT = WS // P              # 2048 free dim per core
FC = 1024                 # chunk of free dim processed at once
NSLOTS = 22               # f32 churn work tiles
NSLOTS16 = 35             # fp16 churn work tiles
NKEEP = 24                # fp16 long-lived tiles (both chunks' w16/t16)
DEG = 3                   # polynomial degree of the quadrature fits

PI = float(np.pi)
TWO_OVER_PI = 0.636619772

# ---- Cody-Waite split of 2*pi ------------------------------------------------


def _split_const(v, bits=12):
    u = np.uint32(np.float32(v).view(np.uint32))
    mask = np.uint32(0xFFFFFFFF) << np.uint32(23 - bits + 1)
    return float(np.uint32(u & mask).view(np.float32))


_2PI = 2 * PI
CW_C1 = _split_const(_2PI)
CW_C2 = _split_const(_2PI - CW_C1)
INV_2PI = float(np.float32(1.0 / _2PI))
MAGIC = 8388608.0

# ---- NR Bessel coefficients (highest degree first) ---------------------------

J0_NUM = [-184.9052456, 77392.33017, -11214424.18, 651619640.7,
          -13362590354.0, 57568490574.0]
J0_DEN = [1.0, 267.8532712, 59272.64853, 9494680.718,
          1029532985.0, 57568490411.0]
J1_NUM = [-30.16036606, 15704.48260, -2972611.439, 242396853.1,
          -7895059235.0, 72362614232.0]
J1_DEN = [1.0, 376.9991397, 99447.43394, 18583304.74,
          2300535178.0, 144725228442.0]
Y0_NUM = [228.4622733, -86327.92757, 10879881.29, -512359803.6,
          7062834065.0, -2957821389.0]
Y0_DEN = [1.0, 226.1030244, 47447.26470, 7189466.438,
          745249964.8, 40076544269.0]
Y1_NUM = [8.511937935e4, -4.237922726e7, 7.349264551e9,
          -5.153438139e11, 1.275274390e13, -4.900604943e13]
Y1_DEN = [1.0, 3.549632885e3, 1.020426050e6, 2.245904002e8,
          3.733650367e10, 4.244419664e12, 2.499580570e14]
P0C = [0.2093887211e-6, -0.2073370639e-5, 0.2734510407e-4,
       -0.1098628627e-2, 1.0]
Q0C = [-0.934935152e-7, 0.7621095161e-6, -0.6911147651e-5,
       0.1430488765e-3, -0.1562499995e-1]
P1C = [-0.240337019e-6, 0.2457520174e-5, -0.3516396496e-4,
       0.183105e-2, 1.0]
Q1C = [0.105787412e-6, -0.88228987e-6, 0.8449199096e-5,
       -0.2002690873e-3, 0.04687499995]

# ---- float64 NR Bessel (fit targets; matches reference.py's definition) ------


def _poly64(y, c):
    acc = np.full_like(y, c[0])
    for cc in c[1:]:
        acc = acc * y + cc
    return acc


def _j0_64(x):
    y = x * x
    small = _poly64(y, J0_NUM) / _poly64(y, J0_DEN)
    z = 8.0 / x
    y2 = z * z
    xx = x - 0.785398164
    big = np.sqrt(TWO_OVER_PI / x) * (np.cos(xx) * _poly64(y2, P0C)
                                      - z * np.sin(xx) * _poly64(y2, Q0C))
    return np.where(x < 8.0, small, big)


def _j1_64(x):
    y = x * x
    small = x * _poly64(y, J1_NUM) / _poly64(y, J1_DEN)
    z = 8.0 / x
    y2 = z * z
    xx = x - 2.356194491
    big = np.sqrt(TWO_OVER_PI / x) * (np.cos(xx) * _poly64(y2, P1C)
                                      - z * np.sin(xx) * _poly64(y2, Q1C))
    return np.where(x < 8.0, small, big)


def _y0_64(x):
    y = x * x
    small = _poly64(y, Y0_NUM) / _poly64(y, Y0_DEN) \
        + TWO_OVER_PI * _j0_64(x) * np.log(x)
    z = 8.0 / x
    y2 = z * z
    xx = x - 0.785398164
    big = np.sqrt(TWO_OVER_PI / x) * (np.sin(xx) * _poly64(y2, P0C)
                                      + z * np.cos(xx) * _poly64(y2, Q0C))
    return np.where(x < 8.0, small, big)


def _y1_64(x):
    y = x * x
    small = x * _poly64(y, Y1_NUM) / _poly64(y, Y1_DEN) \
        + TWO_OVER_PI * (_j1_64(x) * np.log(x) - 1.0 / x)
    z = 8.0 / x
    y2 = z * z
    xx = x - 2.356194491
    big = np.sqrt(TWO_OVER_PI / x) * (np.sin(xx) * _poly64(y2, P1C)
                                      + z * np.cos(xx) * _poly64(y2, Q1C))
    return np.where(x < 8.0, small, big)


def _norm_parts(x):
    """C0,S0 = m0*(cos f0, sin f0); C1,S1 = m1*(cos f1, sin f1): the
    amplitude-normalized Bessel values rotated by the asymptotic phase."""
    amp = np.sqrt(PI * x / 2)
    nJ0, nY0 = amp * _j0_64(x), amp * _y0_64(x)
    nJ1, nY1 = amp * _j1_64(x), amp * _y1_64(x)
    c = np.cos(x - PI / 4)
    s = np.sin(x - PI / 4)
    C0 = nJ0 * c + nY0 * s
    S0 = nY0 * c - nJ0 * s
    c3 = np.cos(x - 3 * PI / 4)
    s3 = np.sin(x - 3 * PI / 4)
    C1 = nJ1 * c3 + nY1 * s3
    S1 = nY1 * c3 - nJ1 * s3
    return C0, S0, C1, S1, nJ0, nY0, nJ1, nY1


def _shell_targets(w, r0, r1):
    x0 = r0 / w
    x1 = r1 / w
    C0a, S0a, C1a, S1a = _norm_parts(x0)[:4]
    C0b, S0b, C1b, S1b = _norm_parts(x1)[:4]
    return dict(
        Pa=C1a * C0b + S1a * S0b, Qa=S0b * C1a - C0b * S1a,
        Pb=w * (C0a * C0b + S0a * S0b), Qb=w * (S0b * C0a - C0b * S0a),
        Pc=C1a * C1b + S1a * S1b, Qc=S1b * C1a - C1b * S1a,
        Pd=C0a * C1b + S0a * S1b, Qd=S1b * C0a - C1b * S0a,
    )


def _boundary_targets(w, r):
    x = r / w
    _, _, _, _, nJ0, nY0, nJ1, nY1 = _norm_parts(x)
    m0sq = nJ0 * nJ0 + nY0 * nY0
    return dict(Fs=-(nJ1 * nJ0 + nY1 * nY0) / m0sq,
                Fc=-(nJ1 * nY0 - nY1 * nJ0) / m0sq)


def _cheb_grid(lo, hi, n):
    k = np.arange(n)
    x = np.cos(PI * (k + 0.5) / n)
    return lo + (hi - lo) * (x + 1) / 2


def _fit(w, y, deg):
    V = np.vander(w, deg + 1)
    coef, *_ = np.linalg.lstsq(V, y, rcond=None)
    return [float(c) for c in coef]


def _plan3(coeffs, dense, target, scale=1.0):
    """Factor the fitted cubic: p = lin(w) * [(w+a)^2 + b] with
    lin = lin_s*w + lin_b; `scale` multiplies the fitted function (fp16
    range headroom; cancels in the final ratio)."""
    if abs(coeffs[0]) < 1e-9 * max(abs(c) for c in coeffs):
        return None
    roots = np.roots(coeffs)
    if len(roots) != 3:
        return None
    ii = int(np.argmin(np.abs(roots.imag)))
    rreal = float(roots[ii].real)
    rest = [roots[j] for j in range(3) if j != ii]
    a = float(-0.5 * (rest[0] + rest[1]).real)
    b = float((rest[0] * rest[1]).real
              - 0.25 * ((rest[0] + rest[1]).real) ** 2)
    c0 = coeffs[0] * scale
    lin_s, lin_b = c0, -c0 * rreal
    f = np.float32
    w = dense.astype(np.float32)
    lin = f(f(lin_s) * w + f(lin_b))
    sq = f(f(w + f(a)) ** 2)
    v = f(f(sq + f(b)) * lin)
    if np.abs(v.astype(np.float64) - target * scale).max() / scale > 1.3e-3:
        return None
    return {'form': '3', 'lin_s': float(lin_s), 'lin_b': float(lin_b),
            'a': float(a), 'b': float(b)}


def _plan4(coeffs, dense, target, scale=1.0):
    """Degree-4 Horner fallback (scale folded into the coefficients)."""
    return {'form': 'A', 'coeffs': [c * scale for c in coeffs]}


SHELL_SCALE = 0.25   # per-shell fit scale; chain scales by 0.25^4, ratio-safe


def fit_all(rho64, wlo, whi, n=2500):
    nodes = _cheb_grid(wlo, whi, n)
    dense = np.linspace(wlo, whi, 8011)
    fits = {}
    for l in range(1, L - 1):
        t = _shell_targets(nodes, rho64[l, 0], rho64[l, 1])
        td = _shell_targets(dense, rho64[l, 0], rho64[l, 1])
        fits[l] = {}
        for k in t:
            p = _plan3(_fit(nodes, t[k], 3), dense, td[k], SHELL_SCALE)
            if p is None:
                p = _plan4(_fit(nodes, t[k], 4), dense, td[k], SHELL_SCALE)
            fits[l][k] = p
    for (l, r) in ((0, rho64[0, 1]), (L - 1, rho64[L - 1, 0])):
        t = _boundary_targets(nodes, r)
        td = _boundary_targets(dense, r)
        fits[l] = {}
        for k in t:
            p = _plan3(_fit(nodes, t[k], 3), dense, td[k], 1.0)
            if p is None:
                p = _plan4(_fit(nodes, t[k], 4), dense, td[k], 1.0)
            fits[l][k] = p
    return fits


# ---- walrus 1-sync-wait-per-instruction workaround --------------------------
_MAXW = 1


def _split_waits(nc):
    for f in nc.m.functions:
        for bb in f.blocks:
            arr = list(bb.instructions)
            out = []
            changed = False
            for mi in arr:
                si = mi.sync_info
                waits = list(si.on_wait) if si is not None and si.on_wait else []
                if len(waits) > _MAXW:
                    changed = True
                    upd = list(si.on_update) if si is not None and si.on_update \
                        else []
                    rest = waits[_MAXW:]
                    for i in range(0, len(rest), _MAXW):
                        ev = nc.engines[mi.engine].nop()
                        cur = nc.cur_bb.bb
                        cur.instructions = [
                            x for x in cur.instructions if x.name != ev.ins.name
                        ]
                        ev.ins.sync_info = bass_rust.SyncInfo(
                            on_wait=rest[i:i + _MAXW], on_update=[])
                        out.append(ev.ins)
                    mi.sync_info = bass_rust.SyncInfo(on_wait=waits[:_MAXW],
                                                      on_update=upd)
                out.append(mi)
            if changed:
                bb.instructions = out


def _patched_drain_and_barrier(self, tick_clock, wait_clock):
    nc = self.nc
    drain_inst = nc.sync.drain()
    wait_clock.add_sem_waits(
        drain_inst.ins, ScopedClock({None: tick_clock.global_clock})
    )
    nc.all_engine_barrier()
    assert self.sems is not None
    popped = nc._tile_sem_poison_stack.pop()
    assert popped is self._sem_poison
    nc.clear_and_free_semaphores(list(self.sems.allocated().values()))
    nc.all_engine_barrier()


tile.TileContext._drain_and_barrier = _patched_drain_and_barrier


def _register_const(nc, *values):
    for v in values:
        v = float(v)
        if (F32, v) in nc.const_aps.aps:
            continue
        t = nc.alloc_sbuf_tensor(f"const-f32-{v}", [128, 1], F32)
        nc.gpsimd.memset(t.ap(), v)
        nc.const_aps.aps[(F32, v)] = t.ap()
    nc.all_engine_barrier()


# ---- kernel emitter ----------------------------------------------------------

# GpSimd shares SBUF ports with the DVE: concurrent Pool tensor_tensor
# slows Vector ~3.5x (measured), so Pool gets no elementwise work at all.
F16 = mybir.dt.float16


def _poly_biases(plan):
    """Square-activation bias constants a plan needs (pre-registered)."""
    if plan['form'] == '3':
        return [float(plan['a'])]
    return []


class Emit:
    def __init__(self, nc, pool):
        self.nc = nc
        self.pool = pool
        self.n = 0

    def t(self):
        self.n += 1
        return self.pool.tile([P, FC], F32, name=f"w{self.n}", tag="w",
                              bufs=NSLOTS)

    def t16(self):
        self.n += 1
        return self.pool.tile([P, FC], F16, name=f"h{self.n}", tag="h",
                              bufs=NSLOTS16)

    def k16(self):
        self.n += 1
        return self.pool.tile([P, FC], F16, name=f"k{self.n}", tag="k",
                              bufs=NKEEP)

    def act(self, a, func, bias=0.0, scale=1.0, out=None):
        out = out if out is not None else self.t()
        self.nc.scalar.activation(out[:], a[:], func, float(bias), float(scale))
        return out

    def affine(self, a, scale, bias):
        return self.act(a, AF.Copy, bias, scale)

    def stt(self, a, s, b, op0, op1, out=None):
        out = out if out is not None else self.t()
        self.nc.vector.scalar_tensor_tensor(out[:], a[:], float(s), b[:],
                                            AL[op0], AL[op1])
        return out

    def tt(self, a, b, op, out=None):
        out = out if out is not None else self.t()
        self.nc.vector.tensor_tensor(out[:], a[:], b[:], AL[op])
        return out

    def tt16(self, a, b, op):
        return self.tt(a, b, op, out=self.t16())

    def poly16(self, w16, plan, mult16):
        """Fitted poly at w16 (fp16) times mult16 (fp16) -> fp16."""
        if plan['form'] == 'A':
            coeffs = plan['coeffs']
            acc = self.act(w16, AF.Copy, 0.0, coeffs[0])
            for c in coeffs[1:-1]:
                acc = self.stt(acc, c, w16, "add", "mult")
            return self.stt(acc, coeffs[-1], mult16, "add", "mult",
                            out=self.t16())
        lin = self.affine(w16, plan['lin_s'], plan['lin_b'])
        sq = self.act(w16, AF.Square, plan['a'], 1.0)
        v = self.stt(sq, plan['b'], lin, "add", "mult", out=self.t16())
        return self.tt16(v, mult16, "mult")


def build(rho32, fits):
    """rho32: float32 [L,2]; fits: per-layer plan dict."""
    nc = bass.Bass()
    consts = [0.0, PI / 2]
    for l in fits:
        for plan in fits[l].values():
            consts.extend(_poly_biases(plan))
    _register_const(nc, *consts)
    om_d = nc.declare_dram_parameter("omega", [P, FT], F32, isOutput=False)
    ek_d = nc.declare_dram_parameter("epsk", [L, P, FT], F32, isOutput=False)
    out_d = {name: nc.declare_dram_parameter(name, [P, FT], F16, isOutput=True)
             for name in ("ren", "imn", "red", "imd")}

    deltas = [float(np.float32(rho32[l, 1]) - np.float32(rho32[l, 0]))
              for l in range(L)]

    FL2PI = float(np.float32(2 * np.pi))
    NCH = FT // FC
    with tile.TileContext(nc) as tc:
        with tc.tile_pool(name="work", bufs=NSLOTS) as pool:
            em = Emit(nc, pool)

            # ---- prologue (ACT-heavy; tables: sqrt -> ln/exp -> trig).
            # For chunk 0 the flexible affines (t16 copy, tr) run on the
            # otherwise-idle Vector engine to shrink the Scalar-bound head;
            # later chunks keep them on Scalar, which idles at the tail. ----
            def prologue(ci):
                on_v = (ci == 0)
                sl = slice(ci * FC, (ci + 1) * FC)
                omega = em.t()
                nc.sync.dma_start(omega[:], om_d[:, sl])
                eps = []
                for l in range(L):
                    e = em.t()
                    nc.sync.dma_start(e[:], ek_d[l, :, sl])
                    eps.append(e)
                se = [em.act(eps[l], AF.Sqrt) for l in range(L)]
                t = [em.tt(omega, se[l], "mult") for l in range(L)]
                lnt = [em.act(t[l], AF.Ln) for l in range(L)]
                w16 = [em.act(lnt[l], AF.Exp, 0.0, -1.0, out=em.k16())
                       for l in range(L)]
                if on_v:
                    t16 = [em.tt(omega, se[l], "mult", out=em.k16())
                           for l in range(L)]
                else:
                    t16 = [em.act(t[l], AF.Copy, out=em.k16())
                           for l in range(L)]
                # rescaled range reduction: u = t - k*(2pi/delta) with
                # k = round(t*delta/2pi); the body computes sin/cos of
                # delta*u via the Sin activation's scale slot.
                u = {}
                for l in range(1, L - 1):
                    dl = deltas[l]
                    if on_v:
                        tr = em.t()
                        nc.vector.tensor_scalar(tr[:], t[l][:], dl * INV_2PI,
                                                MAGIC, AL["mult"], AL["add"])
                    else:
                        tr = em.affine(t[l], dl * INV_2PI, MAGIC)
                    kf2 = em.t()
                    nc.vector.tensor_scalar(kf2[:], tr[:], -MAGIC,
                                            FL2PI / dl, AL["add"], AL["mult"])
                    u[l] = em.tt(t[l], kf2, "subtract")
                return dict(sl=sl, w16=w16, t16=t16, u=u)

            # ---- body (Vector-heavy fp16 block; its ACT ops are Sin once
            # per shell plus Copy/Square, which live in every table) ----
            def body(p, mid=None):
                w16, t16, u = p['w16'], p['t16'], p['u']

                def shell(l):
                    c = fits[l]
                    dl = deltas[l]
                    SD = em.act(u[l], AF.Sin, 0.0, dl, out=em.t16())
                    au = em.act(u[l], AF.Abs)
                    CD = em.act(au, AF.Sin, PI / 2, -dl, out=em.t16())
                    specs = [('Pa', CD), ('Qa', SD), ('Pb', SD),
                             ('Qb', CD), ('Pc', SD), ('Qc', CD),
                             ('Pd', CD), ('Qd', SD)]
                    term = {}
                    for k, trig in specs:
                        term[k] = em.poly16(w16[l], c[k], trig)
                    a = em.tt16(term['Pa'], term['Qa'], "subtract")
                    bt = em.tt16(term['Pb'], term['Qb'], "add")
                    cpre = em.tt16(term['Pc'], term['Qc'], "add")
                    ct = em.tt16(cpre, t16[l], "mult")
                    dd2 = em.tt16(term['Pd'], term['Qd'], "subtract")
                    return a, bt, ct, dd2

                # boundary polys depend only on prologue outputs - hoist
                # them ahead of the chain so the tail is just chain+final
                Ur = em.poly16(w16[0], fits[0]['Fs'], t16[0])
                Ui = em.poly16(w16[0], fits[0]['Fc'], t16[0])
                Vr = em.poly16(w16[L - 1], fits[L - 1]['Fs'], t16[L - 1])
                Vi = em.poly16(w16[L - 1], fits[L - 1]['Fc'], t16[L - 1])

                A, B, C, D = shell(1)
                for l in (2, 3, 4):
                    if l == 3 and mid is not None:
                        mid()      # interleave next chunk's prologue here
                    a, bt, ct, dd2 = shell(l)
                    m1 = em.tt16(A, a, "mult")
                    m2 = em.tt16(B, ct, "mult")
                    A2 = em.tt16(m1, m2, "subtract")
                    m3 = em.tt16(A, bt, "mult")
                    m4 = em.tt16(B, dd2, "mult")
                    B2 = em.tt16(m3, m4, "add")
                    m5 = em.tt16(C, a, "mult")
                    m6 = em.tt16(D, ct, "mult")
                    C2 = em.tt16(m5, m6, "add")
                    m7 = em.tt16(D, dd2, "mult")
                    m8 = em.tt16(C, bt, "mult")
                    D2 = em.tt16(m7, m8, "subtract")
                    A, B, C, D = A2, B2, C2, D2

                # 18-op final: num/den via shared conjugate products
                e_ = em.tt16(Ur, B, "mult")
                f_ = em.tt16(Ui, B, "mult")
                G1r = em.tt16(D, e_, "subtract")
                pa = em.tt16(Vr, G1r, "mult")
                pb = em.tt16(Vi, f_, "mult")
                pc = em.tt16(Vi, G1r, "mult")
                pd = em.tt16(Vr, f_, "mult")
                reVG1 = em.tt16(pa, pb, "add")
                imVG1 = em.tt16(pc, pd, "subtract")
                reVG2 = em.tt16(pa, pb, "subtract")
                imVG2 = em.tt16(pc, pd, "add")
                ua = em.tt16(Ur, A, "mult")
                ub = em.tt16(Ui, A, "mult")
                s1 = em.tt16(C, ua, "add")
                ren = em.tt16(s1, reVG1, "subtract")
                imn = em.tt16(ub, imVG1, "subtract")
                red = em.tt16(s1, reVG2, "subtract")
                imd = em.tt16(ub, imVG2, "add")
                for name, v in (("ren", ren), ("imn", imn),
                                ("red", red), ("imd", imd)):
                    nc.sync.dma_start(out_d[name][:, p['sl']], v[:])

            # P0 B0 P1 B1: Vector enters body-0 as soon as prologue-0 is
            # done while Scalar runs ahead into prologue-1.
            for ci in range(NCH):
                body(prologue(ci))
    _split_waits(nc)
    return nc


# ---- host-side entry ---------------------------------------------------------

_CACHE = {}
TRACE = False
LAST_RESULT = None


def _numpy_fallback(omega, eps, mu, rho):
    """Exact reference math in numpy (mu != 1 path only)."""

    def poly(y, coeffs):
        acc = np.full_like(y, np.float32(coeffs[0]))
        for c2 in coeffs[1:]:
            acc = acc * y + np.float32(c2)
        return acc

    def _j0(x):
        y = x * x
        small = poly(y, J0_NUM) / poly(y, J0_DEN)
        z = np.float32(8.0) / x
        y2 = z * z
        xx = x - np.float32(0.785398164)
        big = np.sqrt(np.float32(TWO_OVER_PI) / x) * (
            np.cos(xx) * poly(y2, P0C) - z * np.sin(xx) * poly(y2, Q0C))
        return np.where(x < 8.0, small, big).astype(np.float32)

    def _j1(x):
        y = x * x
        small = x * poly(y, J1_NUM) / poly(y, J1_DEN)
        z = np.float32(8.0) / x
        y2 = z * z
        xx = x - np.float32(2.356194491)
        big = np.sqrt(np.float32(TWO_OVER_PI) / x) * (
            np.cos(xx) * poly(y2, P1C) - z * np.sin(xx) * poly(y2, Q1C))
        return np.where(x < 8.0, small, big).astype(np.float32)

    def _y0(x):
        y = x * x
        small = poly(y, Y0_NUM) / poly(y, Y0_DEN) + \
            np.float32(TWO_OVER_PI) * _j0(x) * np.log(x)
        z = np.float32(8.0) / x
        y2 = z * z
        xx = x - np.float32(0.785398164)
        big = np.sqrt(np.float32(TWO_OVER_PI) / x) * (
            np.sin(xx) * poly(y2, P0C) + z * np.cos(xx) * poly(y2, Q0C))
        return np.where(x < 8.0, small, big).astype(np.float32)

    def _y1(x):
        y = x * x
        small = x * poly(y, Y1_NUM) / poly(y, Y1_DEN) + \
            np.float32(TWO_OVER_PI) * (_j1(x) * np.log(x) - 1.0 / x)
        z = np.float32(8.0) / x
        y2 = z * z
        xx = x - np.float32(2.356194491)
        big = np.sqrt(np.float32(TWO_OVER_PI) / x) * (
            np.sin(xx) * poly(y2, P1C) + z * np.cos(xx) * poly(y2, Q1C))
        return np.where(x < 8.0, small, big).astype(np.float32)

    omega = omega.astype(np.float32)
    eps = eps.astype(np.float32)
    mu = mu.astype(np.float32)
    k = omega[None, :] * np.sqrt(eps * mu)
    p = np.sqrt(eps / mu)

    def tmat(kl, pl, r0, r1):
        x0 = kl * np.float32(r0)
        x1 = kl * np.float32(r1)
        j_a, y_a = _j0(x0), _y0(x0)
        j_b, y_b = _j0(x1), _y0(x1)
        jd_a, yd_a = -_j1(x0), -_y1(x0)
        jd_b, yd_b = -_j1(x1), -_y1(x1)
        pref = np.float32(PI / 2) * x0
        m00 = (pref * (yd_a * j_b - jd_a * y_b)).astype(np.complex64)
        m01 = (1j / pl) * pref * (j_a * y_b - y_a * j_b)
        m10 = (-1j * pl) * pref * (yd_a * jd_b - jd_a * yd_b)
        m11 = (pref * (j_a * yd_b - y_a * jd_b)).astype(np.complex64)
        return m00, m01, m10, m11

    M00, M01, M10, M11 = tmat(k[1], p[1], rho[1, 0], rho[1, 1])
    for l in range(2, L - 1):
        a, b, c, d = tmat(k[l], p[l], rho[l, 0], rho[l, 1])
        M00, M01, M10, M11 = (M00 * a + M01 * c, M00 * b + M01 * d,
                              M10 * a + M11 * c, M10 * b + M11 * d)

    def cfacs(z):
        j0v, j1v, y0v, y1v = _j0(z), _j1(z), _y0(z), _y1(z)
        c1 = -(j1v + 1j * y1v) / (j0v + 1j * y0v)
        c2 = -(j1v - 1j * y1v) / (j0v - 1j * y0v)
        return c1, c2

    c0_1, c0_2 = cfacs(k[0] * np.float32(rho[0, 1]))
    _, c1_2 = cfacs(k[L - 1] * np.float32(rho[L - 1, 0]))
    p0, p1 = p[0], p[L - 1]
    num = M10 + 1j * p0 * c0_2 * M00 \
        - 1j * p1 * c1_2 * (M11 + 1j * p0 * c0_2 * M01)
    den = -1j * p0 * c0_1 * M00 - M10 \
        - 1j * p1 * c1_2 * (-1j * p0 * c0_1 * M01 - M11)
    r = num / den
    return (r * np.conj(r)).real.astype(np.float32)


def kernel(omega, eps, mu, rho):
    from concourse.bass_utils import run_bass_kernel_spmd

    omega = np.ascontiguousarray(omega, dtype=np.float32)
    eps = np.ascontiguousarray(eps, dtype=np.float32)
    mu = np.ascontiguousarray(mu, dtype=np.float32)
    rho32 = np.asarray(rho, dtype=np.float32)
    assert omega.shape == (W,) and eps.shape == (L, W)

    if not bool(np.all(mu == 1.0)):
        return _numpy_fallback(omega, eps, mu, rho32)

    # fit range from input bounds (w = 1/t, t = omega*sqrt(eps))
    om_min, om_max = float(omega.min()), float(omega.max())
    ep_min, ep_max = float(eps.min()), float(eps.max())
    tmin = om_min * np.sqrt(ep_min)
    tmax = om_max * np.sqrt(ep_max)
    wlo = float(1.0 / (tmax * 1.002))
    whi = float(1.0 / (tmin * 0.998))

    key = (rho32.tobytes(), round(wlo, 5), round(whi, 5), DEG, FC, NSLOTS,
           NSLOTS16, NKEEP, SHELL_SCALE, "v15")
    if key not in _CACHE:
        fits = fit_all(rho32.astype(np.float64), wlo, whi)
        _CACHE[key] = build(rho32, fits)
    nc = _CACHE[key]

    in_maps = []
    for i in range(NCORES):
        sl = slice(i * WS, (i + 1) * WS)
        in_maps.append({"omega": omega[sl].reshape(P, FT),
                        "epsk": eps[:, sl].reshape(L, P, FT)})

    res = run_bass_kernel_spmd(nc, in_maps, core_ids=list(range(NCORES)),
                               trace=TRACE)
    global LAST_RESULT
    LAST_RESULT = res
    out = np.empty((W,), dtype=np.float32)
    for i in range(NCORES):
        r = res.results[i]
        ren = r["ren"].reshape(WS).astype(np.float32)
        imn = r["imn"].reshape(WS).astype(np.float32)
        red = r["red"].reshape(WS).astype(np.float32)
        imd = r["imd"].reshape(WS).astype(np.float32)
        out[i * WS:(i + 1) * WS] = (ren * ren + imn * imn) / \
            (red * red + imd * imd)
    return out
